# revision 20
# baseline (speedup 1.0000x reference)
"""Trainium2 Bass kernel for nn_DecoderLayer_72327249264859.

Decoder layer: self-attn + (cross-attn || graph-attn) + FFN, each with
residual + layernorm. B=4, T=S=1024, D=1024, 16 heads, ffn=4096.

Sharding: pure data-parallel over query tokens. Core c handles batch
element b = c // 2, query rows (c % 2)*512 .. +512. Each core computes
full-length K/V for its batch element (2x duplicated K/V work, but zero
cross-core communication). The host permutes the self-attention key axis
so each core's own query tokens are the first 512 columns of x_t
(attention is invariant to key order as long as the mask rows are
permuted identically), so the same SPMD program works on every core.

Layout strategy: activations are kept *feature-major* in SBUF
([feature-dim on partitions, tokens on free]):
  - V and FC1/FC2 projections run in fp8e4m3 with perf_mode=DoubleRow
    (weights host-scaled by 32, rescaled at PSUM eviction); the k-chunk
    pairs needed by DoubleRow are exactly adjacent chunks of the
    feature-major activation layout, so no relayout is needed. Q/K/O
    projections stay bf16: softmax amplifies score noise and O feeds the
    residual stream directly;
  - scores are computed transposed ([s part, t free]) in bf16, the two
    heads of a pair row-packed into PE row groups 0:64 / 64:128;
  - softmax: exp on ACT; exp(mask) is precomputed on the host and
    multiplied in on DVE; the normalizer Z is accumulated in PSUM with
    col-packed all-ones matmuls per key chunk (no DVE accumulation);
  - layernorm mean / mean-of-squares via (1/D)-ones matmuls landing
    replicated across partitions in PSUM; 1/Z via fast DVE reciprocal.
"""

import sys

if "/opt/trn_rl_repo" not in sys.path:
    sys.path.insert(0, "/opt/trn_rl_repo")

import numpy as np
import ml_dtypes
from contextlib import ExitStack

import concourse.bacc as bacc
import concourse.mybir as mybir
from concourse.tile import TileContext

BF16 = mybir.dt.bfloat16
FP8 = mybir.dt.float8e4
F32 = mybir.dt.float32
AF = mybir.ActivationFunctionType
ALU = mybir.AluOpType
DR = mybir.MatmulPerfMode.DoubleRow

B, T, S, D = 4, 1024, 1024, 1024
NH, DH = 16, 64
F = 4 * D
SCALE = DH**-0.5
EPS = 1e-5
P = 128
KC = D // P        # 8 feature chunks
SC = S // P        # 8 key chunks
FC = F // P        # 32 ffn chunks
TQ = 512           # query tokens per core
NCORES = 8
SW = 32.0          # fp8 weight scale
RSW = 1.0 / SW

# --- per-site precision config (fp8 = DoubleRow matmul) ---
FP8_QK = False     # Q/K projections (score path -- keep bf16)
FP8_V = False       # V projections (self + staged cross/graph)
FP8_O = False      # out projections (residual path)
FP8_FC1 = False
FP8_FC2 = False

# per-partition parameter table layout (each param chunk = one column)
PP_BASE = {}
_off = 0
for _name, _n in (("bq", 24), ("bk", 24), ("bo", 24), ("b1", FC), ("b2", KC),
                  ("g", 32), ("b", 32)):
    PP_BASE[_name] = _off
    _off += _n
PP_COLS = _off

_cache = {}


def build(flags):
    """Builds the per-core Bass program. flags control which (generically
    correct) bias/affine paths get emitted; for the reference inputs all
    biases are zero and gammas are one, so these stay off."""
    nc = bacc.Bacc()
    need_src8 = FP8_V or FP8_QK

    def wdt(fp8):
        return FP8 if fp8 else BF16

    dp = nc.declare_dram_parameter
    x_t = dp("x_t", [D, S], BF16, isOutput=False)
    enc_t = dp("enc_t", [D, S], BF16, isOutput=False)
    gra_t = dp("gra_t", [D, S], BF16, isOutput=False)
    if need_src8:
        x_t8 = dp("x_t8", [D, S], FP8, isOutput=False)
        enc_t8 = dp("enc_t8", [D, S], FP8, isOutput=False)
        gra_t8 = dp("gra_t8", [D, S], FP8, isOutput=False)
    m_self = dp("m_self", [S, TQ], BF16, isOutput=False)
    m_enc = dp("m_enc", [S, TQ], BF16, isOutput=False)
    m_gra = dp("m_gra", [S, TQ], BF16, isOutput=False)
    # weight panels, partition-major: [n_chunk, partition, k_chunk, m]
    wq = [dp(f"wq{a}", [KC, P, KC, P], wdt(FP8_QK), isOutput=False)
          for a in range(3)]
    wk = [dp(f"wk{a}", [KC, P, KC, P], wdt(FP8_QK), isOutput=False)
          for a in range(3)]
    wv = [dp(f"wv{a}", [D, D], wdt(FP8_V), isOutput=False) for a in range(3)]
    wo = [dp(f"wo{a}", [KC, P, KC, P], wdt(FP8_O), isOutput=False)
          for a in range(3)]
    fc1 = dp("fc1", [FC, P, KC, P], wdt(FP8_FC1), isOutput=False)
    fc2 = dp("fc2", [KC, P, FC, P], wdt(FP8_FC2), isOutput=False)
    pp = dp("pp", [P, PP_COLS], F32, isOutput=False)
    bv = dp("bv", [3, D], BF16, isOutput=False)
    out_t = dp("out_t", [D, TQ], F32, isOutput=True)
    # DRAM staging for cross/graph K/V (projected early, read back later)
    kst = [nc.dram_tensor(f"k_st{a}", [D, S], BF16) for a in (1, 2)]
    vst = [nc.dram_tensor(f"v_st{a}", [S, D], BF16) for a in (1, 2)]

    with TileContext(nc) as tc, ExitStack() as ctx:
        const = ctx.enter_context(tc.tile_pool(name="const", bufs=1))
        persist = ctx.enter_context(tc.tile_pool(name="persist", bufs=1))
        srcp = ctx.enter_context(tc.tile_pool(name="srcp", bufs=2))
        maskp = ctx.enter_context(tc.tile_pool(name="maskp", bufs=2))
        kvp = ctx.enter_context(tc.tile_pool(name="kvp", bufs=1))
        wpool = ctx.enter_context(tc.tile_pool(name="wpool", bufs=3))
        wvpool = ctx.enter_context(tc.tile_pool(name="wvpool", bufs=1))
        kvbp = ctx.enter_context(tc.tile_pool(name="kvbp", bufs=2))
        epool = ctx.enter_context(tc.tile_pool(name="epool", bufs=3))
        attnp = ctx.enter_context(tc.tile_pool(name="attnp", bufs=1))
        tmpp = ctx.enter_context(tc.tile_pool(name="tmpp", bufs=2))
        lntp = ctx.enter_context(tc.tile_pool(name="lntp", bufs=2))
        psum = ctx.enter_context(tc.tile_pool(name="psum", bufs=2, space="PSUM"))
        psum1 = ctx.enter_context(tc.tile_pool(name="psum1", bufs=1, space="PSUM"))

        ones = const.tile([P, P], BF16, tag="ones")
        nc.vector.memset(ones, 1.0)
        onesM = const.tile([P, P], BF16, tag="onesM")
        nc.vector.memset(onesM, 1.0 / D)
        epsc = const.tile([P, 1], F32, tag="epsc")
        nc.vector.memset(epsc, EPS)
        pp_sb = const.tile([P, PP_COLS], F32, tag="pp")
        nc.sync.dma_start(pp_sb, pp[:])
        if flags["bv"]:
            bv_sb = const.tile([3, D], BF16, tag="bv")
            nc.sync.dma_start(bv_sb, bv[:])

        def ppc(name, c):
            base = PP_BASE[name]
            return pp_sb[:, base + c: base + c + 1]

        # ---------- helpers ----------
        def mm_chain(ps, wp, rhs_view, k_chunks, fp8, kq=0, kq_split=1):
            """One contraction block into ps. fp8 -> DoubleRow over pairs."""
            if fp8:
                for i in range(k_chunks // 2):
                    nc.tensor.matmul(
                        ps, wp[:, 2 * i:2 * i + 2],
                        rhs_view[:, 2 * i:2 * i + 2],
                        start=(kq == 0 and i == 0),
                        stop=(kq == kq_split - 1 and i == k_chunks // 2 - 1),
                        perf_mode=DR,
                    )
            else:
                for kl in range(k_chunks):
                    nc.tensor.matmul(
                        ps, wp[:, kl], rhs_view[:, kl],
                        start=(kq == 0 and kl == 0),
                        stop=(kq == kq_split - 1 and kl == k_chunks - 1),
                    )

        def proj_fm(dst, w_dram, rhs_sb, n_chunks, k_chunks, Tt,
                    fp8=False, bias=None, relu=False, evict=None, kq_split=1):
            """Feature-major projection: dst[:, ncn, tslice] = panel.T @ rhs.
            kq_split loads the k-dim weight panel in pieces (for big k).
            fp8 evictions rescale by 1/SW for the host-side weight scale."""
            kq_n = k_chunks // kq_split
            assert kq_split == 1 or Tt == 512
            rs = RSW if fp8 else 1.0
            for ncn in range(n_chunks):
                shared_wp = None
                if kq_split == 1:
                    shared_wp = wpool.tile([P, k_chunks, P], wdt(fp8),
                                           tag="wp", name="wp")
                    nc.sync.dma_start(shared_wp, w_dram[ncn])
                for tn in range(Tt // 512):
                    ps = psum.tile([P, 512], F32, tag="proj", name="ps_proj")
                    for kq in range(kq_split):
                        if shared_wp is not None:
                            wp = shared_wp
                        else:
                            wp = wpool.tile([P, kq_n, P], wdt(fp8),
                                            tag="wp", name="wp")
                            nc.sync.dma_start(
                                wp, w_dram[ncn][:, kq * kq_n:(kq + 1) * kq_n]
                            )
                        rv = rhs_sb[:, kq * kq_n:(kq + 1) * kq_n,
                                    tn * 512:(tn + 1) * 512]
                        mm_chain(ps, wp, rv, kq_n, fp8, kq, kq_split)
                    d = dst[:, ncn, tn * 512:(tn + 1) * 512]
                    if evict is not None:
                        evict(ncn, ps, d)
                    elif relu:
                        if bias is not None:
                            nc.vector.tensor_scalar(
                                d, ps, rs, bias(ncn), ALU.mult, ALU.add)
                            nc.vector.tensor_scalar(d, d, 0.0, None, ALU.max)
                        else:
                            nc.vector.tensor_scalar(
                                d, ps, rs, 0.0, ALU.mult, ALU.max)
                    elif bias is not None:
                        nc.vector.tensor_scalar(
                            d, ps, rs, bias(ncn), ALU.mult, ALU.add)
                    elif fp8:
                        nc.vector.tensor_scalar(d, ps, rs, None, ALU.mult)
                    else:
                        nc.scalar.copy(d, ps)

        def proj_tm(v_sb, wv_dram, src_sb, a, wv_sb=None):
            """Token-major V projection: v_sb[:, sc, :]."""
            if wv_sb is None:
                wv_sb = wvpool.tile([P, KC, D], wdt(FP8_V), tag="wv",
                                    name="wv_sb")
                nc.sync.dma_start(wv_sb,
                                  wv_dram.rearrange("(kc p) n -> p kc n", p=P))
            use_bias = flags["bv"]
            for sc in range(SC):
                for n2 in range(2):
                    ps = psum.tile([P, 512], F32, tag="proj", name="ps_v")
                    if FP8_V:
                        for i in range(KC // 2):
                            nc.tensor.matmul(
                                ps,
                                src_sb[:, 2 * i:2 * i + 2, sc * P:(sc + 1) * P],
                                wv_sb[:, 2 * i:2 * i + 2,
                                      n2 * 512:(n2 + 1) * 512],
                                start=(i == 0),
                                stop=(i == KC // 2 - 1 and not use_bias),
                                perf_mode=DR,
                            )
                    else:
                        for kc in range(KC):
                            nc.tensor.matmul(
                                ps,
                                src_sb[:, kc, sc * P:(sc + 1) * P],
                                wv_sb[:, kc, n2 * 512:(n2 + 1) * 512],
                                start=(kc == 0),
                                stop=(kc == KC - 1 and not use_bias),
                            )
                    if use_bias:
                        nc.tensor.matmul(
                            ps, ones[0:1, :] if not FP8_V else ones[0:1, :],
                            bv_sb[a:a + 1, n2 * 512:(n2 + 1) * 512],
                            start=False, stop=True,
                        )
                    d = v_sb[:, sc, n2 * 512:(n2 + 1) * 512]
                    if FP8_V:
                        nc.vector.tensor_scalar(d, ps, RSW, None, ALU.mult)
                    else:
                        nc.scalar.copy(d, ps)

        def stage_groups(a, s_sb, s8_sb):
            """Emit-closures that project attention a's K/V into DRAM
            staging, group by group (each closure = one PSUM group)."""
            groups = []
            dst_k = kst[a - 1].rearrange("(nc p) t -> nc p t", p=P)
            use_kb = flags["bk"]
            rs_k = RSW if FP8_QK else 1.0
            k_src = s8_sb if FP8_QK else s_sb

            # K: share one panel across the 2 tn groups
            panel_cache = {}

            def k_closure(ncn, tn):
                def run():
                    if ncn not in panel_cache:
                        wp = wpool.tile([P, KC, P], wdt(FP8_QK),
                                        tag="wp", name="wp")
                        nc.sync.dma_start(wp, wk[a][ncn])
                        panel_cache[ncn] = wp
                    wp = panel_cache[ncn]
                    ps = psum.tile([P, 512], F32, tag="proj", name="ps_kst")
                    mm_chain(ps, wp, k_src[:, :, tn * 512:(tn + 1) * 512],
                             KC, FP8_QK)
                    bt = kvbp.tile([P, 512], BF16, tag="kvb", name="kvb")
                    if use_kb:
                        nc.vector.tensor_scalar(bt, ps, rs_k,
                                                ppc("bk", a * KC + ncn),
                                                ALU.mult, ALU.add)
                    elif FP8_QK:
                        nc.vector.tensor_scalar(bt, ps, rs_k, None, ALU.mult)
                    else:
                        nc.scalar.copy(bt, ps)
                    nc.sync.dma_start(dst_k[ncn][:, tn * 512:(tn + 1) * 512], bt)

                return run

            for ncn in range(KC):
                for tn in range(2):
                    groups.append(k_closure(ncn, tn))

            dst_v = vst[a - 1].rearrange("(sc p) n -> sc p n", p=P)
            use_vb = flags["bv"]
            v_src = s8_sb if FP8_V else s_sb
            wv_holder = {}

            def v_closure(sc, n2):
                def run():
                    if "wv" not in wv_holder:
                        wv_sb = wvpool.tile([P, KC, D], wdt(FP8_V), tag="wv",
                                            name="wv_sb")
                        nc.sync.dma_start(
                            wv_sb, wv[a].rearrange("(kc p) n -> p kc n", p=P)
                        )
                        wv_holder["wv"] = wv_sb
                    wv_sb = wv_holder["wv"]
                    ps = psum.tile([P, 512], F32, tag="proj", name="ps_vst")
                    if FP8_V:
                        for i in range(KC // 2):
                            nc.tensor.matmul(
                                ps,
                                v_src[:, 2 * i:2 * i + 2, sc * P:(sc + 1) * P],
                                wv_sb[:, 2 * i:2 * i + 2,
                                      n2 * 512:(n2 + 1) * 512],
                                start=(i == 0),
                                stop=(i == KC // 2 - 1 and not use_vb),
                                perf_mode=DR,
                            )
                    else:
                        for kc in range(KC):
                            nc.tensor.matmul(
                                ps,
                                v_src[:, kc, sc * P:(sc + 1) * P],
                                wv_sb[:, kc, n2 * 512:(n2 + 1) * 512],
                                start=(kc == 0),
                                stop=(kc == KC - 1 and not use_vb),
                            )
                    if use_vb:
                        nc.tensor.matmul(
                            ps, ones[0:1, :],
                            bv_sb[a:a + 1, n2 * 512:(n2 + 1) * 512],
                            start=False, stop=True,
                        )
                    bt = kvbp.tile([P, 512], BF16, tag="kvb", name="kvb2")
                    if FP8_V:
                        nc.vector.tensor_scalar(bt, ps, RSW, None, ALU.mult)
                    else:
                        nc.scalar.copy(bt, ps)
                    nc.sync.dma_start(dst_v[sc][:, n2 * 512:(n2 + 1) * 512], bt)

                return run

            for sc in range(SC):
                for n2 in range(2):
                    groups.append(v_closure(sc, n2))
            return groups

        def attention(q_sb, k_sb, v_sb, em_sb, attn_out, filler=None):
            """attn_out[:, hp, :] = softmax-attention, heads in pairs.
            em_sb holds exp(mask) (host-precomputed). Scores row-packed in
            PE row groups; A@V and Z-accumulation col-packed in col groups.
            Z is accumulated in PSUM chunk-by-chunk (no DVE running sum)."""
            for hp in range(NH // 2):
                ps_z = psum1.tile([P, 512], F32, tag="z", name="ps_z")
                ps_o = psum1.tile([P, 512], F32, tag="o", name="ps_o")
                kc_h = hp  # heads 2hp (rows 0:64) and 2hp+1 (rows 64:128)
                for sc in range(SC):
                    # both heads' scores in one 2-bank PSUM tile
                    ps = psum.tile([P, 2, 512], F32, tag="scores", name="ps_sc")
                    for j in range(2):
                        off = j * 64
                        nc.tensor.matmul(
                            ps[:, j],
                            k_sb[off:off + DH, kc_h, sc * P:(sc + 1) * P],
                            q_sb[off:off + DH, kc_h],
                            start=True, stop=True,
                        )
                    e_sb = epool.tile([P, 2, TQ], BF16, tag="e", name="e_sb")
                    nc.scalar.activation(e_sb, ps, AF.Exp)
                    nc.vector.tensor_mul(
                        e_sb, e_sb,
                        em_sb[:, sc:sc + 1, :].to_broadcast((P, 2, TQ)),
                    )
                    for j in range(2):
                        h = 2 * hp + j
                        nc.tensor.matmul(
                            ps_o[j * 64:(j + 1) * 64],
                            v_sb[:, sc, h * DH:(h + 1) * DH], e_sb[:, j],
                            start=(sc == 0), stop=(sc == SC - 1),
                            tile_position=(0, j * 64), skip_group_check=True,
                        )
                        nc.tensor.matmul(
                            ps_z[j * 64:(j + 1) * 64], ones[:, :64], e_sb[:, j],
                            start=(sc == 0), stop=(sc == SC - 1),
                            tile_position=(0, j * 64), skip_group_check=True,
                        )
                rz = tmpp.tile([P, TQ], F32, tag="rz", name="rz")
                nc.vector.reciprocal_approx_fast(rz, ps_z)
                nc.vector.tensor_mul(attn_out[:, hp], ps_o, rz)
                if filler is not None:
                    filler(hp)

        def ln_stats_eager():
            """PSUM stat tiles for eager accumulation during eviction."""
            ps_m = psum1.tile([P, 512], F32, tag="z", name="ps_me")
            ps_s = psum1.tile([P, 512], F32, tag="o", name="ps_se")

            def accum(ncn, d):
                nc.tensor.matmul(ps_m, onesM, d,
                                 start=(ncn == 0), stop=(ncn == KC - 1))
                zsq = lntp.tile([P, TQ], BF16, tag="zsq", name="zsqe")
                nc.vector.tensor_mul(zsq, d, d)
                nc.tensor.matmul(ps_s, onesM, zsq,
                                 start=(ncn == 0), stop=(ncn == KC - 1))

            return (ps_m, ps_s), accum

        def layer_norm(dst_fn, z_sb, li, post=None, stats=None):
            """dst_fn(kc) <- (z - mu) * rstd [* g + b], feature-major.
            post(kc, ap) runs after each chunk is written. stats: eagerly
            pre-accumulated (ps_m, ps_s) from ln_stats_eager()."""
            if stats is not None:
                ps_m, ps_s = stats
            else:
                ps_m = psum1.tile([P, 512], F32, tag="z", name="ps_m")
                for kc in range(KC):
                    nc.tensor.matmul(ps_m, onesM, z_sb[:, kc],
                                     start=(kc == 0), stop=(kc == KC - 1))
                ps_s = psum1.tile([P, 512], F32, tag="o", name="ps_s")
                for kc in range(KC):
                    zsq = lntp.tile([P, TQ], BF16, tag="zsq", name="zsq")
                    nc.vector.tensor_mul(zsq, z_sb[:, kc], z_sb[:, kc])
                    nc.tensor.matmul(ps_s, onesM, zsq,
                                     start=(kc == 0), stop=(kc == KC - 1))
            musq = tmpp.tile([P, TQ], F32, tag="stat", name="musq")
            nc.scalar.square(musq, ps_m)
            var = tmpp.tile([P, TQ], F32, tag="stat", name="var")
            nc.vector.tensor_sub(var, ps_s, musq)
            sd = tmpp.tile([P, TQ], F32, tag="stat", name="sd")
            nc.scalar.activation(sd, var, AF.Sqrt, bias=epsc)
            rstd = tmpp.tile([P, TQ], F32, tag="stat", name="rstd")
            nc.vector.reciprocal_approx_fast(rstd, sd)
            for kc in range(KC):
                t1 = lntp.tile([P, TQ], F32, tag="lnt", name="lnt")
                nc.vector.tensor_sub(t1, z_sb[:, kc], ps_m)
                d = dst_fn(kc)
                if flags["ln_affine"]:
                    t2 = lntp.tile([P, TQ], F32, tag="lnt2", name="lnt2")
                    nc.vector.tensor_mul(t2, t1, rstd)
                    nc.vector.tensor_scalar(
                        d, t2, ppc("g", li * KC + kc), ppc("b", li * KC + kc),
                        ALU.mult, ALU.add,
                    )
                else:
                    nc.vector.tensor_mul(d, t1, rstd)
                if post is not None:
                    post(kc, d)

        # ---------- self attention ----------
        # x arrives in 4 chunk-DMAs (parallel HW queues shorten startup)
        x_sb = srcp.tile([P, KC, S], BF16, tag="src", name="x_sb")
        x_r = x_t.rearrange("(kc p) t -> p kc t", p=P)
        for qd in range(4):
            nc.sync.dma_start(x_sb[:, 2 * qd:2 * qd + 2], x_r[:, 2 * qd:2 * qd + 2])
        if need_src8:
            x8_sb = srcp.tile([P, KC, S], FP8, tag="src8", name="x8_sb")
            nc.sync.dma_start(x8_sb, x_t8.rearrange("(kc p) t -> p kc t", p=P))
        else:
            x8_sb = None

        qk_src = x8_sb if FP8_QK else x_sb
        q_sb = kvp.tile([P, KC, TQ], BF16, tag="Q", name="q0")
        proj_fm(q_sb, wq[0], qk_src[:, :, 0:TQ], KC, KC, TQ, fp8=FP8_QK,
                bias=(lambda c: ppc("bq", c)) if flags["bq"] else None)
        # mask + V-weight DMAs issue here so they hide under the K proj
        em0_sb = maskp.tile([P, SC, TQ], BF16, tag="mask", name="ms_sb")
        nc.sync.dma_start(em0_sb, m_self.rearrange("(sc p) t -> p sc t", p=P))
        wv0_sb = wvpool.tile([P, KC, D], wdt(FP8_V), tag="wv", name="wv_sb")
        nc.sync.dma_start(wv0_sb, wv[0].rearrange("(kc p) n -> p kc n", p=P))
        k_sb = kvp.tile([P, KC, S], BF16, tag="K", name="k0")
        proj_fm(k_sb, wk[0], qk_src, KC, KC, S, fp8=FP8_QK,
                bias=(lambda c: ppc("bk", c)) if flags["bk"] else None)
        v_sb = kvp.tile([P, SC, D], BF16, tag="V", name="v0")
        proj_tm(v_sb, wv[0], x8_sb if FP8_V else x_sb, 0, wv_sb=wv0_sb)

        # cross/graph K/V get projected into DRAM staging; the PE work is
        # interleaved (via fillers) with the ACT-bound attention head loops.
        enc_sb = srcp.tile([P, KC, S], BF16, tag="src", name="src1")
        nc.sync.dma_start(enc_sb, enc_t.rearrange("(kc p) t -> p kc t", p=P))
        if need_src8:
            enc8_sb = srcp.tile([P, KC, S], FP8, tag="src8", name="src81")
            nc.sync.dma_start(enc8_sb,
                              enc_t8.rearrange("(kc p) t -> p kc t", p=P))
        else:
            enc8_sb = None
        enc_groups = stage_groups(1, enc_sb, enc8_sb)

        def filler0(hp):
            n = len(enc_groups)
            for g in enc_groups[hp * n // 8:(hp + 1) * n // 8]:
                g()

        attn0 = attnp.tile([P, KC, TQ], BF16, tag="attn", name="attn0")
        attention(q_sb, k_sb, v_sb, em0_sb, attn0, filler=filler0)

        z1 = persist.tile([P, KC, TQ], BF16, tag="zres", name="z1")

        def evict_res0(ncn, ps, d):
            rs = RSW if FP8_O else 1.0
            if flags["bo"] or FP8_O:
                t = tmpp.tile([P, TQ], F32, tag="evt", name="evt")
                if flags["bo"]:
                    nc.vector.tensor_scalar(t, ps, rs, ppc("bo", ncn),
                                            ALU.mult, ALU.add)
                else:
                    nc.vector.tensor_scalar(t, ps, rs, None, ALU.mult)
                nc.vector.tensor_add(d, t, x_sb[:, ncn, 0:TQ])
            else:
                nc.vector.tensor_add(d, ps, x_sb[:, ncn, 0:TQ])

        proj_fm(z1, wo[0], attn0, KC, KC, TQ, fp8=FP8_O, evict=evict_res0)

        # ---------- cross + graph attention ----------
        # graph source DMA + first staging groups overlap the LN0 window
        # (PE would otherwise idle during the LN stat chain)
        gra_sb = srcp.tile([P, KC, S], BF16, tag="src", name="src2")
        nc.sync.dma_start(gra_sb, gra_t.rearrange("(kc p) t -> p kc t", p=P))
        if need_src8:
            gra8_sb = srcp.tile([P, KC, S], FP8, tag="src8", name="src82")
            nc.sync.dma_start(gra8_sb,
                              gra_t8.rearrange("(kc p) t -> p kc t", p=P))
        else:
            gra8_sb = None
        gra_groups = stage_groups(2, gra_sb, gra8_sb)
        gra_iter = iter(gra_groups)

        h1 = persist.tile([P, KC, TQ], BF16, tag="h1")

        def post_h1(kc, ap):
            if kc % 2 == 0:
                g = next(gra_iter, None)
                if g is not None:
                    g()

        layer_norm(lambda kc: h1[:, kc], z1, 0, post=post_h1)

        h2 = persist.tile([P, KC, TQ], BF16, tag="h2")
        h2f = (persist.tile([P, KC, TQ], FP8, tag="h2f", name="h2f")
               if FP8_FC1 else None)
        for a, m_d in ((1, m_enc), (2, m_gra)):
            if a == 2:
                for g in gra_iter:  # drain any unconsumed staging groups
                    g()
            em_sb = maskp.tile([P, SC, TQ], BF16, tag="mask", name=f"m{a}")
            nc.sync.dma_start(em_sb, m_d.rearrange("(sc p) t -> p sc t", p=P))

            ka = kvp.tile([P, KC, S], BF16, tag="K", name=f"k{a}")
            ka_r = kst[a - 1].rearrange("(kc p) t -> p kc t", p=P)
            va = kvp.tile([P, SC, D], BF16, tag="V", name=f"v{a}")
            va_r = vst[a - 1].rearrange("(sc p) n -> p sc n", p=P)
            for qd in range(4):
                nc.sync.dma_start(ka[:, 2 * qd:2 * qd + 2],
                                  ka_r[:, 2 * qd:2 * qd + 2])
                nc.sync.dma_start(va[:, 2 * qd:2 * qd + 2],
                                  va_r[:, 2 * qd:2 * qd + 2])
            qa = kvp.tile([P, KC, TQ], BF16, tag="Q", name=f"q{a}")
            proj_fm(qa, wq[a], h1, KC, KC, TQ, fp8=False,
                    bias=(lambda c: ppc("bq", a * KC + c)) if flags["bq"] else None)

            if a == 1:
                def filler1(hp):
                    for _ in range(4):
                        g = next(gra_iter, None)
                        if g is not None:
                            g()
            else:
                filler1 = None
            attn_a = attnp.tile([P, KC, TQ], BF16, tag="attn", name=f"attn{a}")
            attention(qa, ka, va, em_sb, attn_a, filler=filler1)

            za = persist.tile([P, KC, TQ], BF16, tag="zres", name=f"za{a}")

            def evict_o(ncn, ps, d, a=a):
                rs = RSW if FP8_O else 1.0
                if flags["bo"]:
                    nc.vector.tensor_scalar(d, ps, rs, ppc("bo", a * KC + ncn),
                                            ALU.mult, ALU.add)
                elif FP8_O:
                    nc.vector.tensor_scalar(d, ps, rs, None, ALU.mult)
                else:
                    nc.scalar.copy(d, ps)

            proj_fm(za, wo[a], attn_a, KC, KC, TQ, fp8=FP8_O, evict=evict_o)
            base = h1 if a == 1 else h2

            def post_add(kc, ap, base=base, last=(a == 2)):
                nc.vector.tensor_add(h2[:, kc], base[:, kc], ap)
                if last and FP8_FC1:
                    nc.vector.tensor_copy(h2f[:, kc], h2[:, kc])

            layer_norm(
                lambda kc: lntp.tile([P, TQ], BF16, tag="lnc", name="lnc"),
                za, a, post=post_add,
            )

        # ---------- FFN ----------
        r_sb = persist.tile([P, FC, TQ], wdt(FP8_FC2), tag="r")
        proj_fm(r_sb, fc1, h2f if FP8_FC1 else h2, FC, KC, TQ, fp8=FP8_FC1,
                bias=(lambda c: ppc("b1", c)) if flags["b1"] else None,
                relu=True)

        z3 = persist.tile([P, KC, TQ], BF16, tag="zres", name="z3")
        stats3, accum3 = ln_stats_eager()

        def evict_fc2(ncn, ps, d):
            rs = RSW if FP8_FC2 else 1.0
            if flags["b2"] or FP8_FC2:
                t = tmpp.tile([P, TQ], F32, tag="evt", name="evt2")
                if flags["b2"]:
                    nc.vector.tensor_scalar(t, ps, rs, ppc("b2", ncn),
                                            ALU.mult, ALU.add)
                else:
                    nc.vector.tensor_scalar(t, ps, rs, None, ALU.mult)
                nc.vector.tensor_add(d, t, h2[:, ncn])
            else:
                nc.vector.tensor_add(d, ps, h2[:, ncn])
            accum3(ncn, d)

        proj_fm(z3, fc2, r_sb, KC, FC, TQ, fp8=FP8_FC2, evict=evict_fc2,
                kq_split=4)

        out_r = out_t.rearrange("(kc p) t -> kc p t", p=P)
        layer_norm(
            lambda kc: lntp.tile([P, TQ], F32, tag="ochunk", name="ochunk"),
            z3, 3, stats=stats3,
            post=lambda kc, ap: nc.sync.dma_start(out_r[kc], ap),
        )

    nc.finalize()
    return nc


def _pp_table(b_q, b_k, b_o, fc1_b, fc2_b, ln_g, ln_b):
    t = np.zeros((P, PP_COLS), np.float32)

    def put(name, vec):
        v = np.asarray(vec, np.float32).reshape(-1, P).T  # [128, n]
        t[:, PP_BASE[name]: PP_BASE[name] + v.shape[1]] = v

    put("bq", b_q.reshape(-1))
    put("bk", b_k.reshape(-1))
    put("bo", b_o.reshape(-1))
    put("b1", fc1_b)
    put("b2", fc2_b)
    put("g", ln_g.reshape(-1))
    put("b", ln_b.reshape(-1))
    return t


def _panels(w):
    """[Din, Dout] -> [Dout//128, 128(p), Din//128, 128(m)] partition-major
    column panels (each SBUF partition line is one contiguous run)."""
    din, dout = w.shape
    return np.ascontiguousarray(
        w.reshape(din // P, P, dout // P, P).transpose(2, 1, 0, 3)
    )


def _bf(a):
    return np.ascontiguousarray(np.asarray(a)).astype(ml_dtypes.bfloat16)


def _f8(a):
    return np.ascontiguousarray(np.asarray(a)).astype(ml_dtypes.float8_e4m3)


def _wcast(w, fp8):
    return _f8(w * SW) if fp8 else _bf(w)


def prepare(inputs):
    """Host-side prep: returns (flags, in_maps)."""
    ii = {k: np.asarray(v, np.float32) for k, v in inputs.items()}
    flags = {
        "bq": bool(np.any(ii["b_q"])),
        "bk": bool(np.any(ii["b_k"])),
        "bv": bool(np.any(ii["b_v"])),
        "bo": bool(np.any(ii["b_o"])),
        "b1": bool(np.any(ii["fc1_b"])),
        "b2": bool(np.any(ii["fc2_b"])),
        "ln_affine": bool(np.any(ii["ln_b"])
                          or not np.allclose(ii["ln_g"], 1.0)),
    }

    pp = _pp_table(ii["b_q"] * SCALE, ii["b_k"], ii["b_o"],
                   ii["fc1_b"], ii["fc2_b"], ii["ln_g"], ii["ln_b"])

    shared = {"fc1": _wcast(_panels(ii["fc1_w"]), FP8_FC1),
              "fc2": _wcast(_panels(ii["fc2_w"]), FP8_FC2),
              "pp": pp, "bv": _bf(ii["b_v"])}
    for a in range(3):
        shared[f"wq{a}"] = _wcast(_panels(ii["W_q"][a] * SCALE), FP8_QK)
        shared[f"wk{a}"] = _wcast(_panels(ii["W_k"][a]), FP8_QK)
        shared[f"wv{a}"] = _wcast(ii["W_v"][a], FP8_V)
        shared[f"wo{a}"] = _wcast(_panels(ii["W_o"][a]), FP8_O)

    hid, enc, gra = (ii["hidden_states"], ii["enc_hidden_states"],
                     ii["graph_hidden_states"])
    msk = [ii["dec_self_mask"], ii["enc_dec_mask"], ii["graph_dec_mask"]]
    emsk = [np.exp(m) for m in msk]
    need_src8 = FP8_V or FP8_QK

    in_maps = []
    for c in range(NCORES):
        b, half = divmod(c, 2)
        r0 = half * TQ
        perm = np.r_[r0:S, 0:r0]  # own tokens first (self-attn key axis)
        m = dict(shared)
        m["x_t"] = _bf(hid[b].T[:, perm])
        m["enc_t"] = _bf(enc[b].T)
        m["gra_t"] = _bf(gra[b].T)
        if need_src8:
            m["x_t8"] = _f8(hid[b].T[:, perm])
            m["enc_t8"] = _f8(enc[b].T)
            m["gra_t8"] = _f8(gra[b].T)
        m["m_self"] = _bf(emsk[0][b, 0].T[perm][:, r0:r0 + TQ])
        m["m_enc"] = _bf(emsk[1][b, 0].T[:, r0:r0 + TQ])
        m["m_gra"] = _bf(emsk[2][b, 0].T[:, r0:r0 + TQ])
        in_maps.append(m)
    return flags, in_maps


def get_program(flags):
    key = tuple(sorted(flags.items()))
    if key not in _cache:
        _cache[key] = build(flags)
    return _cache[key]


def gather(results):
    out = np.zeros((B, T, D), np.float32)
    for c in range(NCORES):
        b, half = divmod(c, 2)
        r0 = half * TQ
        out[b, r0:r0 + TQ, :] = results[c]["out_t"].T
    return out


def kernel(**inputs) -> np.ndarray:
    from concourse.bass_utils import run_bass_kernel_spmd

    flags, in_maps = prepare(inputs)
    nc = get_program(flags)
    res = run_bass_kernel_spmd(nc, in_maps, list(range(NCORES)))
    return gather(res.results)


# revision 23
# speedup vs baseline: 1.8146x; 1.8146x over previous
"""Trainium2 Bass kernel for nn_DecoderLayer_72327249264859.

Decoder layer: self-attn + (cross-attn || graph-attn) + FFN, each with
residual + layernorm. B=4, T=S=1024, D=1024, 16 heads, ffn=4096.

Sharding: pure data-parallel over query tokens. Core c handles batch
element b = c // 2, query rows (c % 2)*512 .. +512. Each core computes
full-length K/V for its batch element (2x duplicated K/V work, but zero
cross-core communication). The host permutes the self-attention key axis
so each core's own query tokens are the first 512 columns of x_t
(attention is invariant to key order as long as the mask rows are
permuted identically), so the same SPMD program works on every core.

Layout strategy: activations are kept *feature-major* in SBUF
([feature-dim on partitions, tokens on free]):
  - V and FC1/FC2 projections run in fp8e4m3 with perf_mode=DoubleRow
    (weights host-scaled by 32, rescaled at PSUM eviction); the k-chunk
    pairs needed by DoubleRow are exactly adjacent chunks of the
    feature-major activation layout, so no relayout is needed. Q/K/O
    projections stay bf16: softmax amplifies score noise and O feeds the
    residual stream directly;
  - scores are computed transposed ([s part, t free]) in bf16, the two
    heads of a pair row-packed into PE row groups 0:64 / 64:128;
  - softmax: exp on ACT; exp(mask) is precomputed on the host and
    multiplied in on DVE; the normalizer Z is accumulated in PSUM with
    col-packed all-ones matmuls per key chunk (no DVE accumulation);
  - layernorm mean / mean-of-squares via (1/D)-ones matmuls landing
    replicated across partitions in PSUM; 1/Z via fast DVE reciprocal.
"""

import sys

if "/opt/trn_rl_repo" not in sys.path:
    sys.path.insert(0, "/opt/trn_rl_repo")

import numpy as np
import ml_dtypes
from contextlib import ExitStack

import concourse.bacc as bacc
import concourse.mybir as mybir
from concourse.tile import TileContext

BF16 = mybir.dt.bfloat16
FP8 = mybir.dt.float8e4
F32 = mybir.dt.float32
AF = mybir.ActivationFunctionType
ALU = mybir.AluOpType
DR = mybir.MatmulPerfMode.DoubleRow

B, T, S, D = 4, 1024, 1024, 1024
NH, DH = 16, 64
F = 4 * D
SCALE = DH**-0.5
EPS = 1e-5
P = 128
KC = D // P        # 8 feature chunks
SC = S // P        # 8 key chunks
FC = F // P        # 32 ffn chunks
TQ = 512           # query tokens per core
NCORES = 8
SW = 32.0          # fp8 weight scale
RSW = 1.0 / SW

# --- per-site precision config (fp8 = DoubleRow matmul) ---
FP8_QK = False     # Q/K projections (score path -- keep bf16)
FP8_V = False       # V projections (self + staged cross/graph)
FP8_O = False      # out projections (residual path)
FP8_FC1 = False
FP8_FC2 = False

# per-partition parameter table layout (each param chunk = one column)
PP_BASE = {}
_off = 0
for _name, _n in (("bq", 24), ("bk", 24), ("bo", 24), ("b1", FC), ("b2", KC),
                  ("g", 32), ("b", 32)):
    PP_BASE[_name] = _off
    _off += _n
PP_COLS = _off

_cache = {}


def build(flags):
    """Builds the per-core Bass program. flags control which (generically
    correct) bias/affine paths get emitted; for the reference inputs all
    biases are zero and gammas are one, so these stay off."""
    nc = bacc.Bacc()
    need_src8 = FP8_V or FP8_QK

    def wdt(fp8):
        return FP8 if fp8 else BF16

    dp = nc.declare_dram_parameter
    x_t = dp("x_t", [D, S], BF16, isOutput=False)
    enc_t = dp("enc_t", [D, S], BF16, isOutput=False)
    gra_t = dp("gra_t", [D, S], BF16, isOutput=False)
    if need_src8:
        x_t8 = dp("x_t8", [D, S], FP8, isOutput=False)
        enc_t8 = dp("enc_t8", [D, S], FP8, isOutput=False)
        gra_t8 = dp("gra_t8", [D, S], FP8, isOutput=False)
    m_self = dp("m_self", [S, TQ], BF16, isOutput=False)
    m_enc = dp("m_enc", [S, TQ], BF16, isOutput=False)
    m_gra = dp("m_gra", [S, TQ], BF16, isOutput=False)
    # weight panels, partition-major: [n_chunk, partition, k_chunk, m]
    wq = [dp(f"wq{a}", [KC, P, KC, P], wdt(FP8_QK), isOutput=False)
          for a in range(3)]
    wk = [dp(f"wk{a}", [KC, P, KC, P], wdt(FP8_QK), isOutput=False)
          for a in range(3)]
    wv = [dp(f"wv{a}", [D, D], wdt(FP8_V), isOutput=False) for a in range(3)]
    wo = [dp(f"wo{a}", [KC, P, KC, P], wdt(FP8_O), isOutput=False)
          for a in range(3)]
    fc1 = dp("fc1", [FC, P, KC, P], wdt(FP8_FC1), isOutput=False)
    fc2 = dp("fc2", [KC, P, FC, P], wdt(FP8_FC2), isOutput=False)
    pp = dp("pp", [P, PP_COLS], F32, isOutput=False)
    bv = dp("bv", [3, D], BF16, isOutput=False)
    out_t = dp("out_t", [D, TQ], F32, isOutput=True)
    # DRAM staging for cross/graph K/V (projected early, read back later)
    kst = [nc.dram_tensor(f"k_st{a}", [D, S], BF16) for a in (1, 2)]
    vst = [nc.dram_tensor(f"v_st{a}", [S, D], BF16) for a in (1, 2)]

    with TileContext(nc) as tc, ExitStack() as ctx:
        const = ctx.enter_context(tc.tile_pool(name="const", bufs=1))
        persist = ctx.enter_context(tc.tile_pool(name="persist", bufs=1))
        srcp = ctx.enter_context(tc.tile_pool(name="srcp", bufs=2))
        maskp = ctx.enter_context(tc.tile_pool(name="maskp", bufs=2))
        kvp = ctx.enter_context(tc.tile_pool(name="kvp", bufs=1))
        wpool = ctx.enter_context(tc.tile_pool(name="wpool", bufs=3))
        wvpool = ctx.enter_context(tc.tile_pool(name="wvpool", bufs=1))
        kvbp = ctx.enter_context(tc.tile_pool(name="kvbp", bufs=2))
        epool = ctx.enter_context(tc.tile_pool(name="epool", bufs=3))
        attnp = ctx.enter_context(tc.tile_pool(name="attnp", bufs=1))
        tmpp = ctx.enter_context(tc.tile_pool(name="tmpp", bufs=2))
        lntp = ctx.enter_context(tc.tile_pool(name="lntp", bufs=2))
        psum = ctx.enter_context(tc.tile_pool(name="psum", bufs=2, space="PSUM"))
        psum1 = ctx.enter_context(tc.tile_pool(name="psum1", bufs=1, space="PSUM"))

        ones = const.tile([P, P], BF16, tag="ones")
        nc.vector.memset(ones, 1.0)
        onesM = const.tile([P, P], BF16, tag="onesM")
        nc.vector.memset(onesM, 1.0 / D)
        epsc = const.tile([P, 1], F32, tag="epsc")
        nc.vector.memset(epsc, EPS)
        pp_sb = const.tile([P, PP_COLS], F32, tag="pp")
        nc.sync.dma_start(pp_sb, pp[:])
        if flags["bv"]:
            bv_sb = const.tile([3, D], BF16, tag="bv")
            nc.sync.dma_start(bv_sb, bv[:])

        def ppc(name, c):
            base = PP_BASE[name]
            return pp_sb[:, base + c: base + c + 1]

        # ---------- helpers ----------
        def mm_chain(ps, wp, rhs_view, k_chunks, fp8, kq=0, kq_split=1):
            """One contraction block into ps. fp8 -> DoubleRow over pairs."""
            if fp8:
                for i in range(k_chunks // 2):
                    nc.tensor.matmul(
                        ps, wp[:, 2 * i:2 * i + 2],
                        rhs_view[:, 2 * i:2 * i + 2],
                        start=(kq == 0 and i == 0),
                        stop=(kq == kq_split - 1 and i == k_chunks // 2 - 1),
                        perf_mode=DR,
                    )
            else:
                for kl in range(k_chunks):
                    nc.tensor.matmul(
                        ps, wp[:, kl], rhs_view[:, kl],
                        start=(kq == 0 and kl == 0),
                        stop=(kq == kq_split - 1 and kl == k_chunks - 1),
                    )

        def proj_fm(dst, w_dram, rhs_sb, n_chunks, k_chunks, Tt,
                    fp8=False, bias=None, relu=False, evict=None, kq_split=1):
            """Feature-major projection: dst[:, ncn, tslice] = panel.T @ rhs.
            kq_split loads the k-dim weight panel in pieces (for big k).
            fp8 evictions rescale by 1/SW for the host-side weight scale."""
            kq_n = k_chunks // kq_split
            assert kq_split == 1 or Tt == 512
            rs = RSW if fp8 else 1.0
            for ncn in range(n_chunks):
                shared_wp = None
                if kq_split == 1:
                    shared_wp = wpool.tile([P, k_chunks, P], wdt(fp8),
                                           tag="wp", name="wp")
                    nc.sync.dma_start(shared_wp, w_dram[ncn])
                for tn in range(Tt // 512):
                    ps = psum.tile([P, 512], F32, tag="proj", name="ps_proj")
                    for kq in range(kq_split):
                        if shared_wp is not None:
                            wp = shared_wp
                        else:
                            wp = wpool.tile([P, kq_n, P], wdt(fp8),
                                            tag="wp", name="wp")
                            nc.sync.dma_start(
                                wp, w_dram[ncn][:, kq * kq_n:(kq + 1) * kq_n]
                            )
                        rv = rhs_sb[:, kq * kq_n:(kq + 1) * kq_n,
                                    tn * 512:(tn + 1) * 512]
                        mm_chain(ps, wp, rv, kq_n, fp8, kq, kq_split)
                    d = dst[:, ncn, tn * 512:(tn + 1) * 512]
                    if evict is not None:
                        evict(ncn, ps, d)
                    elif relu:
                        if bias is not None:
                            nc.vector.tensor_scalar(
                                d, ps, rs, bias(ncn), ALU.mult, ALU.add)
                            nc.vector.tensor_scalar(d, d, 0.0, None, ALU.max)
                        else:
                            nc.vector.tensor_scalar(
                                d, ps, rs, 0.0, ALU.mult, ALU.max)
                    elif bias is not None:
                        nc.vector.tensor_scalar(
                            d, ps, rs, bias(ncn), ALU.mult, ALU.add)
                    elif fp8:
                        nc.vector.tensor_scalar(d, ps, rs, None, ALU.mult)
                    else:
                        nc.scalar.copy(d, ps)

        def proj_tm(v_sb, wv_dram, src_sb, a, wv_sb=None):
            """Token-major V projection: v_sb[:, sc, :]."""
            if wv_sb is None:
                wv_sb = wvpool.tile([P, KC, D], wdt(FP8_V), tag="wv",
                                    name="wv_sb")
                nc.sync.dma_start(wv_sb,
                                  wv_dram.rearrange("(kc p) n -> p kc n", p=P))
            use_bias = flags["bv"]
            for sc in range(SC):
                for n2 in range(2):
                    ps = psum.tile([P, 512], F32, tag="proj", name="ps_v")
                    if FP8_V:
                        for i in range(KC // 2):
                            nc.tensor.matmul(
                                ps,
                                src_sb[:, 2 * i:2 * i + 2, sc * P:(sc + 1) * P],
                                wv_sb[:, 2 * i:2 * i + 2,
                                      n2 * 512:(n2 + 1) * 512],
                                start=(i == 0),
                                stop=(i == KC // 2 - 1 and not use_bias),
                                perf_mode=DR,
                            )
                    else:
                        for kc in range(KC):
                            nc.tensor.matmul(
                                ps,
                                src_sb[:, kc, sc * P:(sc + 1) * P],
                                wv_sb[:, kc, n2 * 512:(n2 + 1) * 512],
                                start=(kc == 0),
                                stop=(kc == KC - 1 and not use_bias),
                            )
                    if use_bias:
                        nc.tensor.matmul(
                            ps, ones[0:1, :] if not FP8_V else ones[0:1, :],
                            bv_sb[a:a + 1, n2 * 512:(n2 + 1) * 512],
                            start=False, stop=True,
                        )
                    d = v_sb[:, sc, n2 * 512:(n2 + 1) * 512]
                    if FP8_V:
                        nc.vector.tensor_scalar(d, ps, RSW, None, ALU.mult)
                    else:
                        nc.scalar.copy(d, ps)

        def stage_groups(a, s_sb, s8_sb):
            """Emit-closures that project attention a's K/V into DRAM
            staging, group by group (each closure = one PSUM group)."""
            groups = []
            dst_k = kst[a - 1].rearrange("(nc p) t -> nc p t", p=P)
            use_kb = flags["bk"]
            rs_k = RSW if FP8_QK else 1.0
            k_src = s8_sb if FP8_QK else s_sb

            # K: share one panel across the 2 tn groups
            panel_cache = {}

            def k_closure(ncn, tn):
                def run():
                    if ncn not in panel_cache:
                        wp = wpool.tile([P, KC, P], wdt(FP8_QK),
                                        tag="wp", name="wp")
                        nc.sync.dma_start(wp, wk[a][ncn])
                        panel_cache[ncn] = wp
                    wp = panel_cache[ncn]
                    ps = psum.tile([P, 512], F32, tag="proj", name="ps_kst")
                    mm_chain(ps, wp, k_src[:, :, tn * 512:(tn + 1) * 512],
                             KC, FP8_QK)
                    bt = kvbp.tile([P, 512], BF16, tag="kvb", name="kvb")
                    if use_kb:
                        nc.vector.tensor_scalar(bt, ps, rs_k,
                                                ppc("bk", a * KC + ncn),
                                                ALU.mult, ALU.add)
                    elif FP8_QK:
                        nc.vector.tensor_scalar(bt, ps, rs_k, None, ALU.mult)
                    else:
                        nc.scalar.copy(bt, ps)
                    nc.sync.dma_start(dst_k[ncn][:, tn * 512:(tn + 1) * 512], bt)

                return run

            for ncn in range(KC):
                for tn in range(2):
                    groups.append(k_closure(ncn, tn))

            dst_v = vst[a - 1].rearrange("(sc p) n -> sc p n", p=P)
            use_vb = flags["bv"]
            v_src = s8_sb if FP8_V else s_sb
            wv_holder = {}

            def v_closure(sc, n2):
                def run():
                    if "wv" not in wv_holder:
                        wv_sb = wvpool.tile([P, KC, D], wdt(FP8_V), tag="wv",
                                            name="wv_sb")
                        nc.sync.dma_start(
                            wv_sb, wv[a].rearrange("(kc p) n -> p kc n", p=P)
                        )
                        wv_holder["wv"] = wv_sb
                    wv_sb = wv_holder["wv"]
                    ps = psum.tile([P, 512], F32, tag="proj", name="ps_vst")
                    if FP8_V:
                        for i in range(KC // 2):
                            nc.tensor.matmul(
                                ps,
                                v_src[:, 2 * i:2 * i + 2, sc * P:(sc + 1) * P],
                                wv_sb[:, 2 * i:2 * i + 2,
                                      n2 * 512:(n2 + 1) * 512],
                                start=(i == 0),
                                stop=(i == KC // 2 - 1 and not use_vb),
                                perf_mode=DR,
                            )
                    else:
                        for kc in range(KC):
                            nc.tensor.matmul(
                                ps,
                                v_src[:, kc, sc * P:(sc + 1) * P],
                                wv_sb[:, kc, n2 * 512:(n2 + 1) * 512],
                                start=(kc == 0),
                                stop=(kc == KC - 1 and not use_vb),
                            )
                    if use_vb:
                        nc.tensor.matmul(
                            ps, ones[0:1, :],
                            bv_sb[a:a + 1, n2 * 512:(n2 + 1) * 512],
                            start=False, stop=True,
                        )
                    bt = kvbp.tile([P, 512], BF16, tag="kvb", name="kvb2")
                    if FP8_V:
                        nc.vector.tensor_scalar(bt, ps, RSW, None, ALU.mult)
                    else:
                        nc.scalar.copy(bt, ps)
                    nc.sync.dma_start(dst_v[sc][:, n2 * 512:(n2 + 1) * 512], bt)

                return run

            for sc in range(SC):
                for n2 in range(2):
                    groups.append(v_closure(sc, n2))
            return groups

        def attention(q_sb, k_sb, v_sb, em_sb, attn_out, filler=None):
            """attn_out[:, hp, :] = softmax-attention, heads in pairs.
            em_sb holds exp(mask) (host-precomputed). Scores row-packed in
            PE row groups; A@V and Z-accumulation col-packed in col groups.
            Z is accumulated in PSUM chunk-by-chunk (no DVE running sum)."""
            for hp in range(NH // 2):
                ps_z = psum1.tile([P, 512], F32, tag="z", name="ps_z")
                ps_o = psum1.tile([P, 512], F32, tag="o", name="ps_o")
                kc_h = hp  # heads 2hp (rows 0:64) and 2hp+1 (rows 64:128)
                for sc in range(SC):
                    # both heads' scores in one 2-bank PSUM tile
                    ps = psum.tile([P, 2, 512], F32, tag="scores", name="ps_sc")
                    for j in range(2):
                        off = j * 64
                        nc.tensor.matmul(
                            ps[:, j],
                            k_sb[off:off + DH, kc_h, sc * P:(sc + 1) * P],
                            q_sb[off:off + DH, kc_h],
                            start=True, stop=True,
                        )
                    e_sb = epool.tile([P, 2, TQ], BF16, tag="e", name="e_sb")
                    nc.scalar.activation(e_sb, ps, AF.Exp)
                    nc.vector.tensor_mul(
                        e_sb, e_sb,
                        em_sb[:, sc:sc + 1, :].to_broadcast((P, 2, TQ)),
                    )
                    for j in range(2):
                        h = 2 * hp + j
                        nc.tensor.matmul(
                            ps_o[j * 64:(j + 1) * 64],
                            v_sb[:, sc, h * DH:(h + 1) * DH], e_sb[:, j],
                            start=(sc == 0), stop=(sc == SC - 1),
                            tile_position=(0, j * 64), skip_group_check=True,
                        )
                        nc.tensor.matmul(
                            ps_z[j * 64:(j + 1) * 64], ones[:, :64], e_sb[:, j],
                            start=(sc == 0), stop=(sc == SC - 1),
                            tile_position=(0, j * 64), skip_group_check=True,
                        )
                rz = tmpp.tile([P, TQ], F32, tag="rz", name="rz")
                nc.vector.reciprocal_approx_fast(rz, ps_z)
                nc.vector.tensor_mul(attn_out[:, hp], ps_o, rz)
                if filler is not None:
                    filler(hp)

        def ln_stats_eager():
            """PSUM stat tiles for eager accumulation during eviction."""
            ps_m = psum1.tile([P, 512], F32, tag="z", name="ps_me")
            ps_s = psum1.tile([P, 512], F32, tag="o", name="ps_se")

            def accum(ncn, d):
                nc.tensor.matmul(ps_m, onesM, d,
                                 start=(ncn == 0), stop=(ncn == KC - 1))
                zsq = lntp.tile([P, TQ], BF16, tag="zsq", name="zsqe")
                nc.vector.tensor_mul(zsq, d, d)
                nc.tensor.matmul(ps_s, onesM, zsq,
                                 start=(ncn == 0), stop=(ncn == KC - 1))

            return (ps_m, ps_s), accum

        def layer_norm(dst_fn, z_sb, li, post=None, stats=None):
            """dst_fn(kc) <- (z - mu) * rstd [* g + b], feature-major.
            post(kc, ap) runs after each chunk is written. stats: eagerly
            pre-accumulated (ps_m, ps_s) from ln_stats_eager()."""
            if stats is not None:
                ps_m, ps_s = stats
            else:
                ps_m = psum1.tile([P, 512], F32, tag="z", name="ps_m")
                for kc in range(KC):
                    nc.tensor.matmul(ps_m, onesM, z_sb[:, kc],
                                     start=(kc == 0), stop=(kc == KC - 1))
                ps_s = psum1.tile([P, 512], F32, tag="o", name="ps_s")
                for kc in range(KC):
                    zsq = lntp.tile([P, TQ], BF16, tag="zsq", name="zsq")
                    nc.vector.tensor_mul(zsq, z_sb[:, kc], z_sb[:, kc])
                    nc.tensor.matmul(ps_s, onesM, zsq,
                                     start=(kc == 0), stop=(kc == KC - 1))
            musq = tmpp.tile([P, TQ], F32, tag="stat", name="musq")
            nc.scalar.square(musq, ps_m)
            var = tmpp.tile([P, TQ], F32, tag="stat", name="var")
            nc.vector.tensor_sub(var, ps_s, musq)
            sd = tmpp.tile([P, TQ], F32, tag="stat", name="sd")
            nc.scalar.activation(sd, var, AF.Sqrt, bias=epsc)
            rstd = tmpp.tile([P, TQ], F32, tag="stat", name="rstd")
            nc.vector.reciprocal_approx_fast(rstd, sd)
            for kc in range(KC):
                t1 = lntp.tile([P, TQ], F32, tag="lnt", name="lnt")
                nc.vector.tensor_sub(t1, z_sb[:, kc], ps_m)
                d = dst_fn(kc)
                if flags["ln_affine"]:
                    t2 = lntp.tile([P, TQ], F32, tag="lnt2", name="lnt2")
                    nc.vector.tensor_mul(t2, t1, rstd)
                    nc.vector.tensor_scalar(
                        d, t2, ppc("g", li * KC + kc), ppc("b", li * KC + kc),
                        ALU.mult, ALU.add,
                    )
                else:
                    nc.vector.tensor_mul(d, t1, rstd)
                if post is not None:
                    post(kc, d)

        # ---------- self attention ----------
        # x arrives in 4 chunk-DMAs (parallel HW queues shorten startup)
        x_sb = srcp.tile([P, KC, S], BF16, tag="src", name="x_sb")
        x_r = x_t.rearrange("(kc p) t -> p kc t", p=P)
        for qd in range(4):
            nc.sync.dma_start(x_sb[:, 2 * qd:2 * qd + 2], x_r[:, 2 * qd:2 * qd + 2])
        if need_src8:
            x8_sb = srcp.tile([P, KC, S], FP8, tag="src8", name="x8_sb")
            nc.sync.dma_start(x8_sb, x_t8.rearrange("(kc p) t -> p kc t", p=P))
        else:
            x8_sb = None

        qk_src = x8_sb if FP8_QK else x_sb
        q_sb = kvp.tile([P, KC, TQ], BF16, tag="Q", name="q0")
        proj_fm(q_sb, wq[0], qk_src[:, :, 0:TQ], KC, KC, TQ, fp8=FP8_QK,
                bias=(lambda c: ppc("bq", c)) if flags["bq"] else None)
        # mask + V-weight DMAs issue here so they hide under the K proj
        em0_sb = maskp.tile([P, SC, TQ], BF16, tag="mask", name="ms_sb")
        nc.sync.dma_start(em0_sb, m_self.rearrange("(sc p) t -> p sc t", p=P))
        wv0_sb = wvpool.tile([P, KC, D], wdt(FP8_V), tag="wv", name="wv_sb")
        nc.sync.dma_start(wv0_sb, wv[0].rearrange("(kc p) n -> p kc n", p=P))
        k_sb = kvp.tile([P, KC, S], BF16, tag="K", name="k0")
        proj_fm(k_sb, wk[0], qk_src, KC, KC, S, fp8=FP8_QK,
                bias=(lambda c: ppc("bk", c)) if flags["bk"] else None)
        v_sb = kvp.tile([P, SC, D], BF16, tag="V", name="v0")
        proj_tm(v_sb, wv[0], x8_sb if FP8_V else x_sb, 0, wv_sb=wv0_sb)

        # cross/graph K/V get projected into DRAM staging; the PE work is
        # interleaved (via fillers) with the ACT-bound attention head loops.
        enc_sb = srcp.tile([P, KC, S], BF16, tag="src", name="src1")
        nc.sync.dma_start(enc_sb, enc_t.rearrange("(kc p) t -> p kc t", p=P))
        if need_src8:
            enc8_sb = srcp.tile([P, KC, S], FP8, tag="src8", name="src81")
            nc.sync.dma_start(enc8_sb,
                              enc_t8.rearrange("(kc p) t -> p kc t", p=P))
        else:
            enc8_sb = None
        enc_groups = stage_groups(1, enc_sb, enc8_sb)

        def filler0(hp):
            n = len(enc_groups)
            for g in enc_groups[hp * n // 8:(hp + 1) * n // 8]:
                g()

        attn0 = attnp.tile([P, KC, TQ], BF16, tag="attn", name="attn0")
        attention(q_sb, k_sb, v_sb, em0_sb, attn0, filler=filler0)

        z1 = persist.tile([P, KC, TQ], BF16, tag="zres", name="z1")
        stats0, accum0 = ln_stats_eager()

        def evict_res0(ncn, ps, d):
            rs = RSW if FP8_O else 1.0
            if flags["bo"] or FP8_O:
                t = tmpp.tile([P, TQ], F32, tag="evt", name="evt")
                if flags["bo"]:
                    nc.vector.tensor_scalar(t, ps, rs, ppc("bo", ncn),
                                            ALU.mult, ALU.add)
                else:
                    nc.vector.tensor_scalar(t, ps, rs, None, ALU.mult)
                nc.vector.tensor_add(d, t, x_sb[:, ncn, 0:TQ])
            else:
                nc.vector.tensor_add(d, ps, x_sb[:, ncn, 0:TQ])
            accum0(ncn, d)

        proj_fm(z1, wo[0], attn0, KC, KC, TQ, fp8=FP8_O, evict=evict_res0)

        # ---------- cross + graph attention ----------
        # graph source DMA + first staging groups overlap the LN0 window
        # (PE would otherwise idle during the LN stat chain)
        gra_sb = srcp.tile([P, KC, S], BF16, tag="src", name="src2")
        nc.sync.dma_start(gra_sb, gra_t.rearrange("(kc p) t -> p kc t", p=P))
        if need_src8:
            gra8_sb = srcp.tile([P, KC, S], FP8, tag="src8", name="src82")
            nc.sync.dma_start(gra8_sb,
                              gra_t8.rearrange("(kc p) t -> p kc t", p=P))
        else:
            gra8_sb = None
        gra_groups = stage_groups(2, gra_sb, gra8_sb)
        gra_iter = iter(gra_groups)

        h1 = persist.tile([P, KC, TQ], BF16, tag="h1")

        def post_h1(kc, ap):
            if kc % 2 == 0:
                g = next(gra_iter, None)
                if g is not None:
                    g()

        layer_norm(lambda kc: h1[:, kc], z1, 0, post=post_h1, stats=stats0)

        h2 = persist.tile([P, KC, TQ], BF16, tag="h2")
        h2f = (persist.tile([P, KC, TQ], FP8, tag="h2f", name="h2f")
               if FP8_FC1 else None)
        for a, m_d in ((1, m_enc), (2, m_gra)):
            if a == 2:
                for g in gra_iter:  # drain any unconsumed staging groups
                    g()
            em_sb = maskp.tile([P, SC, TQ], BF16, tag="mask", name=f"m{a}")
            nc.sync.dma_start(em_sb, m_d.rearrange("(sc p) t -> p sc t", p=P))

            ka = kvp.tile([P, KC, S], BF16, tag="K", name=f"k{a}")
            ka_r = kst[a - 1].rearrange("(kc p) t -> p kc t", p=P)
            va = kvp.tile([P, SC, D], BF16, tag="V", name=f"v{a}")
            va_r = vst[a - 1].rearrange("(sc p) n -> p sc n", p=P)
            for qd in range(4):
                nc.sync.dma_start(ka[:, 2 * qd:2 * qd + 2],
                                  ka_r[:, 2 * qd:2 * qd + 2])
                nc.sync.dma_start(va[:, 2 * qd:2 * qd + 2],
                                  va_r[:, 2 * qd:2 * qd + 2])
            qa = kvp.tile([P, KC, TQ], BF16, tag="Q", name=f"q{a}")
            proj_fm(qa, wq[a], h1, KC, KC, TQ, fp8=False,
                    bias=(lambda c: ppc("bq", a * KC + c)) if flags["bq"] else None)

            if a == 1:
                def filler1(hp):
                    for _ in range(4):
                        g = next(gra_iter, None)
                        if g is not None:
                            g()
            else:
                filler1 = None
            attn_a = attnp.tile([P, KC, TQ], BF16, tag="attn", name=f"attn{a}")
            attention(qa, ka, va, em_sb, attn_a, filler=filler1)

            za = persist.tile([P, KC, TQ], BF16, tag="zres", name=f"za{a}")
            stats_a, accum_a = ln_stats_eager()

            def evict_o(ncn, ps, d, a=a, accum_a=accum_a):
                rs = RSW if FP8_O else 1.0
                if flags["bo"]:
                    nc.vector.tensor_scalar(d, ps, rs, ppc("bo", a * KC + ncn),
                                            ALU.mult, ALU.add)
                elif FP8_O:
                    nc.vector.tensor_scalar(d, ps, rs, None, ALU.mult)
                else:
                    nc.scalar.copy(d, ps)
                accum_a(ncn, d)

            proj_fm(za, wo[a], attn_a, KC, KC, TQ, fp8=FP8_O, evict=evict_o)
            base = h1 if a == 1 else h2

            def post_add(kc, ap, base=base, last=(a == 2)):
                nc.vector.tensor_add(h2[:, kc], base[:, kc], ap)
                if last and FP8_FC1:
                    nc.vector.tensor_copy(h2f[:, kc], h2[:, kc])

            layer_norm(
                lambda kc: lntp.tile([P, TQ], BF16, tag="lnc", name="lnc"),
                za, a, post=post_add, stats=stats_a,
            )

        # ---------- FFN ----------
        r_sb = persist.tile([P, FC, TQ], wdt(FP8_FC2), tag="r")
        proj_fm(r_sb, fc1, h2f if FP8_FC1 else h2, FC, KC, TQ, fp8=FP8_FC1,
                bias=(lambda c: ppc("b1", c)) if flags["b1"] else None,
                relu=True)

        z3 = persist.tile([P, KC, TQ], BF16, tag="zres", name="z3")
        stats3, accum3 = ln_stats_eager()

        def evict_fc2(ncn, ps, d):
            rs = RSW if FP8_FC2 else 1.0
            if flags["b2"] or FP8_FC2:
                t = tmpp.tile([P, TQ], F32, tag="evt", name="evt2")
                if flags["b2"]:
                    nc.vector.tensor_scalar(t, ps, rs, ppc("b2", ncn),
                                            ALU.mult, ALU.add)
                else:
                    nc.vector.tensor_scalar(t, ps, rs, None, ALU.mult)
                nc.vector.tensor_add(d, t, h2[:, ncn])
            else:
                nc.vector.tensor_add(d, ps, h2[:, ncn])
            accum3(ncn, d)

        proj_fm(z3, fc2, r_sb, KC, FC, TQ, fp8=FP8_FC2, evict=evict_fc2,
                kq_split=4)

        out_r = out_t.rearrange("(kc p) t -> kc p t", p=P)
        layer_norm(
            lambda kc: lntp.tile([P, TQ], F32, tag="ochunk", name="ochunk"),
            z3, 3, stats=stats3,
            post=lambda kc, ap: nc.sync.dma_start(out_r[kc], ap),
        )

    nc.finalize()
    return nc


def _pp_table(b_q, b_k, b_o, fc1_b, fc2_b, ln_g, ln_b):
    t = np.zeros((P, PP_COLS), np.float32)

    def put(name, vec):
        v = np.asarray(vec, np.float32).reshape(-1, P).T  # [128, n]
        t[:, PP_BASE[name]: PP_BASE[name] + v.shape[1]] = v

    put("bq", b_q.reshape(-1))
    put("bk", b_k.reshape(-1))
    put("bo", b_o.reshape(-1))
    put("b1", fc1_b)
    put("b2", fc2_b)
    put("g", ln_g.reshape(-1))
    put("b", ln_b.reshape(-1))
    return t


def _panels(w):
    """[Din, Dout] -> [Dout//128, 128(p), Din//128, 128(m)] partition-major
    column panels (each SBUF partition line is one contiguous run)."""
    din, dout = w.shape
    return np.ascontiguousarray(
        w.reshape(din // P, P, dout // P, P).transpose(2, 1, 0, 3)
    )


def _bf(a):
    return np.ascontiguousarray(np.asarray(a)).astype(ml_dtypes.bfloat16)


def _f8(a):
    return np.ascontiguousarray(np.asarray(a)).astype(ml_dtypes.float8_e4m3)


def _wcast(w, fp8):
    return _f8(w * SW) if fp8 else _bf(w)


def prepare(inputs):
    """Host-side prep: returns (flags, in_maps)."""
    ii = {k: np.asarray(v, np.float32) for k, v in inputs.items()}
    flags = {
        "bq": bool(np.any(ii["b_q"])),
        "bk": bool(np.any(ii["b_k"])),
        "bv": bool(np.any(ii["b_v"])),
        "bo": bool(np.any(ii["b_o"])),
        "b1": bool(np.any(ii["fc1_b"])),
        "b2": bool(np.any(ii["fc2_b"])),
        "ln_affine": bool(np.any(ii["ln_b"])
                          or not np.allclose(ii["ln_g"], 1.0)),
    }

    pp = _pp_table(ii["b_q"] * SCALE, ii["b_k"], ii["b_o"],
                   ii["fc1_b"], ii["fc2_b"], ii["ln_g"], ii["ln_b"])

    shared = {"fc1": _wcast(_panels(ii["fc1_w"]), FP8_FC1),
              "fc2": _wcast(_panels(ii["fc2_w"]), FP8_FC2),
              "pp": pp, "bv": _bf(ii["b_v"])}
    for a in range(3):
        shared[f"wq{a}"] = _wcast(_panels(ii["W_q"][a] * SCALE), FP8_QK)
        shared[f"wk{a}"] = _wcast(_panels(ii["W_k"][a]), FP8_QK)
        shared[f"wv{a}"] = _wcast(ii["W_v"][a], FP8_V)
        shared[f"wo{a}"] = _wcast(_panels(ii["W_o"][a]), FP8_O)

    hid, enc, gra = (ii["hidden_states"], ii["enc_hidden_states"],
                     ii["graph_hidden_states"])
    msk = [ii["dec_self_mask"], ii["enc_dec_mask"], ii["graph_dec_mask"]]
    emsk = [np.exp(m) for m in msk]
    need_src8 = FP8_V or FP8_QK

    in_maps = []
    for c in range(NCORES):
        b, half = divmod(c, 2)
        r0 = half * TQ
        perm = np.r_[r0:S, 0:r0]  # own tokens first (self-attn key axis)
        m = dict(shared)
        m["x_t"] = _bf(hid[b].T[:, perm])
        m["enc_t"] = _bf(enc[b].T)
        m["gra_t"] = _bf(gra[b].T)
        if need_src8:
            m["x_t8"] = _f8(hid[b].T[:, perm])
            m["enc_t8"] = _f8(enc[b].T)
            m["gra_t8"] = _f8(gra[b].T)
        m["m_self"] = _bf(emsk[0][b, 0].T[perm][:, r0:r0 + TQ])
        m["m_enc"] = _bf(emsk[1][b, 0].T[:, r0:r0 + TQ])
        m["m_gra"] = _bf(emsk[2][b, 0].T[:, r0:r0 + TQ])
        in_maps.append(m)
    return flags, in_maps


def get_program(flags):
    key = tuple(sorted(flags.items()))
    if key not in _cache:
        _cache[key] = build(flags)
    return _cache[key]


def gather(results):
    out = np.zeros((B, T, D), np.float32)
    for c in range(NCORES):
        b, half = divmod(c, 2)
        r0 = half * TQ
        out[b, r0:r0 + TQ, :] = results[c]["out_t"].T
    return out


def kernel(**inputs) -> np.ndarray:
    from concourse.bass_utils import run_bass_kernel_spmd

    flags, in_maps = prepare(inputs)
    nc = get_program(flags)
    res = run_bass_kernel_spmd(nc, in_maps, list(range(NCORES)))
    return gather(res.results)
